# revision 23
# baseline (speedup 1.0000x reference)
"""Trainium2 Bass kernel for the per-sample dynamic-depthwise-conv block.

Computation (per sample b):
    att  = sigmoid(lrelu(v @ ca_w1.T) @ ca_w2.T)            # [b, 64]
    kern = (lrelu(v @ k_w1.T) @ k_w2.T).reshape(b*64,1,3,3) # per-(b,c) 3x3
    y    = lrelu(depthwise3x3(x0 * att, kern))
    out  = conv1x1(y, conv_w) + conv_b

Strategy: data-parallel over batch across 8 cores (4 samples/core).  On each
core, samples are processed in 2 "pairs"; a pair's 2x64 channels fill the 128
SBUF partitions.  The attention gate is folded into the generated tap weights
(dw(att*x) == att*dw(x) per channel).  x is pre-padded on the host to width
130 (zero side columns) so each input DMA moves full 4160-byte contiguous
row blocks (no sub-512B descriptor penalty); the vertical one-row borders are
zeroed in SBUF.

The 9 depthwise taps are split across all four compute engines to balance
busy time (PE is the scarce resource):
  - PE: taps (0,1),(1,1),(2,1),(1,2) on all rows plus (2,0) on rows 0-63,
    as PSUM-accumulated matmuls with diagonal bf16 lhsT against shifted
    views of the padded tile (4.5 tap-passes), plus one identity-matmul
    injection of the SBUF partial per 8-row group, plus the final 1x1 conv.
  - DVE: taps (0,0),(0,2),(1,0) (even column offsets keep the 4x/2x bf16
    modes aligned) accumulated into 32-row-block partials, plus adds of
    the gpsimd products.
  - GpSimd(Pool): tap (2,2) on all rows and (2,0) on rows 64-127 as
    tensor_scalar products into SBUF chunks that DVE folds into the partial.
Leaky-relu runs on the Scalar engine (Prelu, PSUM->SBUF, bf16) over
[128,1024] groups; the final 1x1 conv is one matmul per 8-row group with a
block-diagonal [conv_w.T, conv_w.T] lhsT; conv bias rides the Scalar
engine's Identity activation during the PSUM->SBUF copy; outputs leave as
bf16 and are widened to fp32 on the host.

The tiny MLP + per-pair tap-scalar prep is software-pipelined one iteration
ahead (ping-pong A/B diag buffer sets, 2x-unrolled hardware loop) so the
MLP->gather->diag chain never sits on the PE critical path at the loop seam.
"""

import sys

if "/opt/trn_rl_repo" not in sys.path:
    sys.path.append("/opt/trn_rl_repo")

import numpy as np
import ml_dtypes

B, C, H, W = 32, 64, 128, 128
KK = 3
RED = 8
N_CORES = 8
BPC = B // N_CORES          # samples per core (4)
PAIRS = BPC // 2            # sample pairs per core (2)
HP, WP = H + 2, W + 2       # padded image dims (130); width padded on host
RPG = 8                     # output rows per PE group -> N = 8*128 = 1024
NGRP = H // RPG             # 16 groups per pair
BLK = 32                    # DVE partial block rows
NBLK = H // BLK             # 4 blocks per pair
QCH = 32                    # gpsimd product chunk rows

# tap assignment
DVE_TAPS = [(0, 0), (0, 2), (1, 0), (2, 2)]  # even dj -> aligned
NO_POOL = True
PE_TAPS = [(0, 1), (1, 1), (2, 1), (1, 2)]
HALF_TAP = (2, 0)                       # rows 0-63 on PE, rest on gpsimd
POOL_TAP = (2, 2)                       # all rows on gpsimd
PE_ROWS_SPLIT = 128                     # first rows of HALF_TAP on PE

_CACHE = {}


class _Env:
    pass


def _build(repeat=1, n_taps=9, unroll=None):
    import concourse.bass as bass  # noqa: F401
    from concourse import bacc, tile, mybir

    f32 = mybir.dt.float32
    bf16 = mybir.dt.bfloat16

    e = _Env()
    e.mybir = mybir
    e.AF = mybir.ActivationFunctionType
    e.f32 = f32
    e.bf16 = bf16

    nc = bacc.Bacc(None, target_bir_lowering=False, debug=False)
    e.nc = nc

    e.x = nc.dram_tensor("x", [BPC, C, H, WP], bf16, kind="ExternalInput")
    e.vt = nc.dram_tensor("vt", [C, BPC], f32, kind="ExternalInput")
    e.caw1t = nc.dram_tensor("caw1t", [C, RED], f32, kind="ExternalInput")
    e.caw2t = nc.dram_tensor("caw2t", [RED, C], f32, kind="ExternalInput")
    e.kw1t = nc.dram_tensor("kw1t", [C, C], f32, kind="ExternalInput")
    e.kw2t = nc.dram_tensor("kw2t", [C, C * KK * KK], f32,
                            kind="ExternalInput")
    e.convt = nc.dram_tensor("convt", [128, 128], bf16, kind="ExternalInput")
    e.bcol = nc.dram_tensor("bcol", [128, 1], f32, kind="ExternalInput")
    e.eye = nc.dram_tensor("eye", [128, 128], f32, kind="ExternalInput")
    e.eyebf = nc.dram_tensor("eyebf", [128, 128], bf16, kind="ExternalInput")
    e.out = nc.dram_tensor("out", [BPC, C, H, W], bf16, kind="ExternalOutput")

    with tile.TileContext(nc) as tc:
        with (
            tc.tile_pool(name="consts", bufs=1) as consts,
            tc.tile_pool(name="stage", bufs=1) as stage,
            tc.tile_pool(name="diags", bufs=1) as diags,
            tc.tile_pool(name="xin", bufs=2) as xin,
            tc.tile_pool(name="parts", bufs=3) as parts,
            tc.tile_pool(name="qs", bufs=4) as qs,
            tc.tile_pool(name="ys", bufs=4) as ys,
            tc.tile_pool(name="os", bufs=4) as osb,
            tc.tile_pool(name="psA", bufs=2, space="PSUM") as psA,
            tc.tile_pool(name="psB", bufs=2, space="PSUM") as psB,
        ):
            e.consts, e.stage, e.diags = consts, stage, diags
            e.xin, e.parts, e.qs, e.ys, e.osb = xin, parts, qs, ys, osb
            e.psA, e.psB = psA, psB

            # ---- constants into SBUF ----
            e.vt_sb = consts.tile([C, BPC], f32)
            e.caw1t_sb = consts.tile([C, RED], f32)
            e.caw2t_sb = consts.tile([RED, C], f32)
            e.kw1t_sb = consts.tile([C, C], f32)
            e.kw2t_sb = consts.tile([C, C * KK * KK], f32)
            e.convt_sb = consts.tile([128, 128], bf16)
            e.bcol_sb = consts.tile([128, 1], f32)
            e.eye_sb = consts.tile([128, 128], f32)
            e.eyebf_sb = consts.tile([128, 128], bf16)
            for t, d in (
                (e.vt_sb, e.vt), (e.caw1t_sb, e.caw1t),
                (e.caw2t_sb, e.caw2t), (e.kw1t_sb, e.kw1t),
                (e.kw2t_sb, e.kw2t), (e.convt_sb, e.convt),
                (e.bcol_sb, e.bcol), (e.eye_sb, e.eye),
                (e.eyebf_sb, e.eyebf),
            ):
                nc.sync.dma_start(out=t[:], in_=d.ap())

            e.xv = e.x.ap().rearrange("(pr s) c h w -> pr (s c) h w",
                                      pr=PAIRS)
            e.ov = e.out.ap().rearrange("(pr s) c h w -> pr (s c) h w",
                                        pr=PAIRS)

            setA = _alloc_bufset(e, "A")
            setB = _alloc_bufset(e, "B")

            _prep(e, setA)
            if repeat == 1:
                n_un = unroll or 1
                sets = [setA, setB]
                for u in range(n_un):
                    stp = (_prep_steps(e, sets[(u + 1) % 2])
                           if u + 1 < n_un else None)
                    _main(e, sets[u % 2], stp, variant=n_taps)
            else:
                assert unroll is None
                half, odd = divmod(repeat, 2)
                if half > 0:
                    with tc.For_i(0, half, 1):
                        _main(e, setA, _prep_steps(e, setB), variant=n_taps)
                        _main(e, setB, _prep_steps(e, setA), variant=n_taps)
                if odd:
                    _main(e, setA, variant=n_taps)

    nc.compile()
    return nc


def _alloc_bufset(e, nm):
    """Per-iteration tap-scalar buffers: diag tiles for the PE taps and the
    d-column tile per pair."""
    s = _Env()
    s.diag = [{} for _ in range(PAIRS)]
    s.dcols = []
    for pr in range(PAIRS):
        s.dcols.append(
            e.stage.tile([128, KK * KK], e.f32, tag=f"d{nm}{pr}",
                         name=f"d{nm}{pr}"))
        for (di, dj) in PE_TAPS + [HALF_TAP]:
            t = di * KK + dj
            s.diag[pr][(di, dj)] = e.diags.tile(
                [128, 128], e.bf16, tag=f"diag{nm}{pr}_{t}",
                name=f"diag{nm}{pr}_{t}")
    return s


def _prep_steps(e, bset):
    """MLP + per-pair tap-scalar prep as a list of emit-closures so the
    caller can interleave them between main-loop groups (hides the
    matmul->activation round-trip latencies behind main-loop work)."""
    nc, AF, f32 = e.nc, e.AF, e.f32
    NK = C * KK * KK
    st = _Env()
    steps = []

    def s_h1():
        ps_h1 = e.psA.tile([RED, BPC], f32, tag="pa", name="ps_h1")
        nc.tensor.matmul(ps_h1[:], lhsT=e.caw1t_sb[:], rhs=e.vt_sb[:],
                         start=True, stop=True)
        st.h1t = e.stage.tile([RED, BPC], f32, tag="h1t", name="h1t")
        nc.scalar.activation(st.h1t[:], ps_h1[:], AF.Prelu, alpha=0.1)

    def s_att():
        ps_att = e.psA.tile([C, BPC], f32, tag="pa", name="ps_att")
        nc.tensor.matmul(ps_att[:], lhsT=e.caw2t_sb[:], rhs=st.h1t[:],
                         start=True, stop=True)
        st.att = e.stage.tile([C, BPC], f32, tag="att", name="att")
        nc.scalar.activation(st.att[:], ps_att[:], AF.Sigmoid)

    def s_h2():
        ps_h2 = e.psA.tile([C, BPC], f32, tag="pa", name="ps_h2")
        nc.tensor.matmul(ps_h2[:], lhsT=e.kw1t_sb[:], rhs=e.vt_sb[:],
                         start=True, stop=True)
        st.h2t = e.stage.tile([C, BPC], f32, tag="h2t", name="h2t")
        nc.scalar.activation(st.h2t[:], ps_h2[:], AF.Prelu, alpha=0.1)

    def s_kern():
        ps_k = e.psA.tile([BPC, NK], f32, tag="pa", name="ps_k")
        nc.tensor.matmul(ps_k[:, 0:512], lhsT=st.h2t[:],
                         rhs=e.kw2t_sb[:, 0:512], start=True, stop=True)
        nc.tensor.matmul(ps_k[:, 512:NK], lhsT=st.h2t[:],
                         rhs=e.kw2t_sb[:, 512:NK], start=True, stop=True)
        st.kern = e.stage.tile([BPC, NK], f32, tag="kern", name="kern")
        nc.scalar.activation(st.kern[:], ps_k[:], AF.Copy)

    def s_gather():
        st.dtap = e.stage.tile([128, PAIRS, KK * KK], f32, tag="dtap",
                               name="dtap")
        st.attpp = e.stage.tile([128, PAIRS], f32, tag="attpp",
                                name="attpp")
        for pr in range(PAIRS):
            for sdx in range(2):
                b = pr * 2 + sdx
                src = st.kern[b:b + 1, :].rearrange(
                    "o (c t) -> o c t", c=C)
                nc.scalar.dma_start(
                    out=st.dtap[C * sdx:C * (sdx + 1), pr:pr + 1, :],
                    in_=src)
                nc.scalar.dma_start(
                    out=st.attpp[C * sdx:C * (sdx + 1), pr:pr + 1],
                    in_=st.att[:, b:b + 1])

    def s_dcol(pr):
        def f():
            nc.vector.tensor_scalar_mul(
                bset.dcols[pr][:], st.dtap[:, pr, :],
                st.attpp[:, pr:pr + 1])
        return f

    def s_diag(pr, di, dj):
        def f():
            t = di * KK + dj
            nc.scalar.activation(
                bset.diag[pr][(di, dj)][:], e.eye_sb[:], AF.Copy,
                scale=bset.dcols[pr][:, t:t + 1])
        return f

    steps = [s_h1, s_att, s_h2, s_kern, s_gather]
    for pr in range(PAIRS):
        steps.append(s_dcol(pr))
        for (di, dj) in PE_TAPS + [HALF_TAP]:
            steps.append(s_diag(pr, di, dj))
    return steps


def _prep(e, bset):
    for f in _prep_steps(e, bset):
        f()


def _main(e, bset, steps=None, variant=9):
    """Two-pair main loop reading tap scalars from `bset`.  `steps` are
    next-iteration prep emitters, drained one per PE group."""
    nc, AF, f32, bf16 = e.nc, e.AF, e.f32, e.bf16
    steps = list(steps) if steps else []

    for pr in range(PAIRS):
        xt = e.xin.tile([128, HP, WP], bf16, tag="xt")
        # zero the top/bottom border rows (side columns are zero from the
        # host-side width padding)
        nc.gpsimd.memset(xt[:, 0, :], 0.0)
        nc.gpsimd.memset(xt[:, HP - 1, :], 0.0)
        # full-width contiguous row-block loads (4160 B per partition)
        nsplit = 8
        rstep = H // nsplit
        for k in (range(nsplit) if variant != 1 else []):
            r0 = k * rstep
            nc.sync.dma_start(
                out=xt[:, 1 + r0:1 + r0 + rstep, :],
                in_=e.xv[pr, :, r0:r0 + rstep, :])

        # gpsimd tap products, in QCH-row chunks
        q22 = {}
        q20 = {}
        t22 = POOL_TAP[0] * KK + POOL_TAP[1]
        t20 = HALF_TAP[0] * KK + HALF_TAP[1]
        for qc in ([] if NO_POOL else range(H // QCH)):
            r0 = qc * QCH
            qt = e.qs.tile([128, QCH, W], bf16, tag="q22", name=f"q22_{qc}")
            nc.gpsimd.tensor_scalar_mul(
                qt[:],
                xt[:, r0 + POOL_TAP[0]:r0 + POOL_TAP[0] + QCH,
                   POOL_TAP[1]:POOL_TAP[1] + W],
                bset.dcols[pr][:, t22:t22 + 1])
            q22[qc] = qt
        for qc in range(PE_ROWS_SPLIT // QCH, H // QCH):
            r0 = qc * QCH
            qt = e.qs.tile([128, QCH, W], bf16, tag="q20", name=f"q20_{qc}")
            nc.gpsimd.tensor_scalar_mul(
                qt[:],
                xt[:, r0 + HALF_TAP[0]:r0 + HALF_TAP[0] + QCH,
                   HALF_TAP[1]:HALF_TAP[1] + W],
                bset.dcols[pr][:, t20:t20 + 1])
            q20[qc] = qt

        # DVE partial blocks (BLK rows each)
        dve_taps = list(DVE_TAPS)
        if variant == 8:
            dve_taps = dve_taps[:-1]
        part_of = {}
        for b in ([] if variant == 5 else range(NBLK)):
            r0 = b * BLK
            part = e.parts.tile([128, BLK, W], bf16, tag="part",
                                bufs=8, name=f"part{b}")
            part_of[b] = part
            for n, (di, dj) in enumerate(dve_taps):
                t = di * KK + dj
                xin_v = xt[:, r0 + di:r0 + di + BLK, dj:dj + W]
                if n == 0:
                    nc.vector.tensor_scalar_mul(
                        part[:], xin_v, bset.dcols[pr][:, t:t + 1])
                else:
                    tmp = e.parts.tile([128, BLK, W], bf16, tag="tmp",
                                       name=f"tmp{b}_{n}")
                    nc.vector.tensor_scalar_mul(
                        tmp[:], xin_v, bset.dcols[pr][:, t:t + 1])
                    nc.vector.tensor_add(part[:], part[:], tmp[:])
            # fold in the gpsimd products (QCH == BLK row chunks)
            for h in range(0 if NO_POOL else BLK // QCH):
                qc = b * (BLK // QCH) + h
                sl = part[:, h * QCH:(h + 1) * QCH, :]
                nc.vector.tensor_add(sl, sl, q22[qc][:])
                if qc in q20:
                    nc.vector.tensor_add(sl, sl, q20[qc][:])

        # PE groups: taps + injection -> lrelu -> conv -> bias -> store
        for g in range(NGRP):
            i0 = g * RPG
            taps = list(PE_TAPS)
            if variant == 7:
                taps = taps[:-1]
            if i0 < PE_ROWS_SPLIT:
                taps.append(HALF_TAP)
            pa = e.psA.tile([128, RPG * W], f32, tag="pa", name=f"pa{g}")
            HB = RPG // 2  # rows per 512-col sub-chunk (one PSUM bank)
            NW = HB * W
            for t_idx, (di, dj) in enumerate(taps):
                dg = bset.diag[pr][(di, dj)]
                last = (variant in (5, 6)) and t_idx == len(taps) - 1
                for c2 in range(2):
                    j0 = i0 + c2 * HB
                    nc.tensor.matmul(
                        pa[:, c2 * NW:(c2 + 1) * NW],
                        lhsT=dg[:],
                        rhs=xt[:, j0 + di:j0 + di + HB, dj:dj + W],
                        start=(t_idx == 0), stop=last,
                        skip_group_check=True)
            if variant not in (5, 6):
                part = part_of[i0 // BLK]
                roff = i0 % BLK
                for c2 in range(2):
                    nc.tensor.matmul(
                        pa[:, c2 * NW:(c2 + 1) * NW], lhsT=e.eyebf_sb[:],
                        rhs=part[:, roff + c2 * HB:roff + c2 * HB + HB, :],
                        start=False, stop=True, skip_group_check=True)

            yt = e.ys.tile([128, RPG * W], bf16, tag="yt")
            nc.scalar.activation(yt[:], pa[:], AF.Prelu, alpha=0.1)

            if variant == 4:
                nc.sync.dma_start(
                    out=e.ov[pr, :, i0:i0 + RPG, :],
                    in_=yt[:].rearrange("p (r w) -> p r w", r=RPG))
                continue
            pb = e.psB.tile([128, RPG * W], f32, tag="pb")
            for c2 in range(2):
                nc.tensor.matmul(pb[:, c2 * NW:(c2 + 1) * NW],
                                 lhsT=e.convt_sb[:],
                                 rhs=yt[:, c2 * NW:(c2 + 1) * NW],
                                 start=True, stop=True)
            ot = e.osb.tile([128, RPG * W], bf16, tag="ot")
            nc.scalar.activation(ot[:], pb[:], AF.Identity,
                                 bias=e.bcol_sb[:, 0:1])
            if variant != 2:
                nc.sync.dma_start(
                    out=e.ov[pr, :, i0:i0 + RPG, :],
                    in_=ot[:].rearrange("p (r w) -> p r w", r=RPG))
            if steps and (pr * NGRP + g) >= 2:
                steps.pop(0)()
    while steps:
        steps.pop(0)()


def get_nc(repeat=1, n_taps=9, unroll=None):
    key = ("nc", repeat, n_taps, unroll)
    if key not in _CACHE:
        _CACHE[key] = _build(repeat, n_taps, unroll)
    return _CACHE[key]


def make_in_maps(x0, v, ca_w1, ca_w2, k_w1, k_w2, conv_w, conv_b):
    bf = ml_dtypes.bfloat16
    caw1t = np.ascontiguousarray(ca_w1.T, dtype=np.float32)
    caw2t = np.ascontiguousarray(ca_w2.T, dtype=np.float32)
    kw1t = np.ascontiguousarray(k_w1.T, dtype=np.float32)
    kw2t = np.ascontiguousarray(k_w2.T, dtype=np.float32)
    convt = np.zeros((128, 128), dtype=bf)
    cwt = conv_w.T.astype(bf)
    convt[0:64, 0:64] = cwt
    convt[64:128, 64:128] = cwt
    bcol = np.tile(conv_b.astype(np.float32), 2)[:, None].copy()
    eye = np.eye(128, dtype=np.float32)
    eyebf = np.eye(128, dtype=bf)
    xpad = np.zeros((B, C, H, WP), dtype=bf)
    xpad[:, :, :, 1:1 + W] = np.asarray(x0)
    in_maps = []
    for k in range(N_CORES):
        sl = slice(k * BPC, (k + 1) * BPC)
        in_maps.append({
            "x": np.ascontiguousarray(xpad[sl]),
            "vt": np.ascontiguousarray(v[sl].T, dtype=np.float32),
            "caw1t": caw1t, "caw2t": caw2t, "kw1t": kw1t, "kw2t": kw2t,
            "convt": convt, "bcol": bcol, "eye": eye, "eyebf": eyebf,
        })
    return in_maps


def kernel(x0, v, ca_w1, ca_w2, k_w1, k_w2, conv_w, conv_b):
    from concourse.bass_utils import run_bass_kernel_spmd

    nc = get_nc()
    in_maps = make_in_maps(x0, v, ca_w1, ca_w2, k_w1, k_w2, conv_w, conv_b)
    res = run_bass_kernel_spmd(nc, in_maps, list(range(N_CORES)))
    return np.concatenate([res.results[i]["out"] for i in range(N_CORES)],
                          axis=0).astype(np.float32)
